# revision 18
# baseline (speedup 1.0000x reference)
"""Causal self-attention with RoPE for trn2, 8-core SPMD.

Problem (hardcoded): B=2, T=2048, C=1024, 16 heads, head_dim=64, fp32 io.
  qkv = x @ w_attn.T; q,k roped; causal softmax(q k^T/8) v; y @ w_proj.T

Sharding: core c -> (batch b = c//4, head-group g = c%4) — 4 heads per core.
Each core computes its group's partial output projection; host sums the 4
group partials per batch.

Device layout (per core):
  xT   [C, T]  f32  — x[b] transposed on host (feeds matmul contraction dim)
  wqkT [C, 512] f32 — [Wq_g | Wk_g] transposed (cols: 4 heads x 64 q, then k)
  wvT  [C, 260] f32 — Wv_g transposed, padded: per head 64 cols + 1 zero col
                      (the zero col becomes the "ones" column for sum-exp)
  wpT  [256, C] bf16 — w_proj[:, group cols] transposed
  cosT/sinT [128, T] bf16 — RoPE tables transposed, 2-head stacked; sinT rows
                      0:32/64:96 pre-negated so rope = q*cos + swap32(q)*sin
  out  [T, C]  f32  — partial output (needs host sum over the 4 groups)

Pipeline: QK^T projection in float32r (w stationary -> transposed layout,
  RoPE on DVE) + V projection in float32r (x stationary -> natural layout)
  -> per head-pair flash attention: S^T tiles on PE (row-packed pairs), exp
  on ACT straight from PSUM, causal mask via gpsimd affine_select after exp,
  then [V|1] matmul (M=65) accumulates y^T and sum-exp together in PSUM;
  normalize via reciprocal_approx + broadcast-DMA; final projection back to
  natural layout. Bulk DMA on the sync-engine HWDGE ring.
"""

from contextlib import ExitStack

import numpy as np
import ml_dtypes

import concourse.bass as bass
import concourse.tile as tile
from concourse import bacc, mybir
from concourse.bass_utils import run_bass_kernel_spmd

B, T, C = 2, 2048, 1024
NH, HD = 16, 64
HG = 4              # heads per group (per core)
GD = HG * HD        # 256
NCC = C // 128      # 8 contraction chunks
F32 = mybir.dt.float32
F32R = mybir.dt.float32r
BF16 = mybir.dt.bfloat16
BF = ml_dtypes.bfloat16

QB = 512            # query block size
KT = 128            # key tile size


def build_kernel(t=T):
    nc = bacc.Bacc("TRN2", target_bir_lowering=False, debug=False)
    xT = nc.dram_tensor("xT", [C, t], F32, kind="ExternalInput").ap()
    wqkT = nc.dram_tensor("wqkT", [C, 2 * GD], BF16, kind="ExternalInput").ap()
    wvT = nc.dram_tensor("wvT", [C, HG * (HD + 1)], BF16,
                         kind="ExternalInput").ap()
    wpT = nc.dram_tensor("wpT", [GD, C], BF16, kind="ExternalInput").ap()
    cosT = nc.dram_tensor("cosT", [128, t], BF16, kind="ExternalInput").ap()
    sinT = nc.dram_tensor("sinT", [128, t], BF16, kind="ExternalInput").ap()
    out = nc.dram_tensor("out", [t, C], F32, kind="ExternalOutput").ap()

    with tile.TileContext(nc) as tc:
        _attn_body(tc, out, xT, wqkT, wvT, wpT, cosT, sinT, t)
    nc.compile()
    return nc


def _attn_body(tc, out, xT, wqkT, wvT, wpT, cosT, sinT, t):
    ctx = ExitStack()
    nc = tc.nc
    ntt = t // 128          # t tiles (and k tiles)
    nqb = t // QB           # query blocks
    ntb = t // QB           # t blocks for projections
    Exp = mybir.ActivationFunctionType.Exp
    Log = mybir.ActivationFunctionType.Ln
    ge = mybir.AluOpType.is_ge

    consts = ctx.enter_context(tc.tile_pool(name="consts", bufs=1))
    resident = ctx.enter_context(tc.tile_pool(name="resident", bufs=1))
    ropet = ctx.enter_context(tc.tile_pool(name="ropet", bufs=3))
    exps = ctx.enter_context(tc.tile_pool(name="exps", bufs=6))
    small = ctx.enter_context(tc.tile_pool(name="small", bufs=2))
    outsb = ctx.enter_context(tc.tile_pool(name="outsb", bufs=3))
    psA = ctx.enter_context(tc.tile_pool(name="psA", bufs=2, space="PSUM"))
    psS = ctx.enter_context(tc.tile_pool(name="psS", bufs=2, space="PSUM"))
    psY = ctx.enter_context(tc.tile_pool(name="psY", bufs=2, space="PSUM"))

    # ---- constants in ----
    cos_sb = consts.tile([128, t], BF16)
    nc.sync.dma_start(cos_sb[:], cosT[:])
    sin_sb = consts.tile([128, t], BF16)
    nc.sync.dma_start(sin_sb[:], sinT[:])
    wqk_sb = consts.tile([128, NCC, 2 * GD], BF16)
    nc.sync.dma_start(wqk_sb[:], wqkT.rearrange("(cc p) j -> p cc j", p=128))
    wv_sb = consts.tile([128, NCC, HG * (HD + 1)], BF16)
    nc.sync.dma_start(wv_sb[:], wvT.rearrange("(cc p) j -> p cc j", p=128))
    wp_sb = consts.tile([128, 2, C], BF16)
    nc.sync.dma_start(wp_sb[:], wpT.rearrange("(jc p) c -> p jc c", p=128))

    # ---- x^T load f32 + DVE cast to bf16 ----
    xstage = ctx.enter_context(tc.tile_pool(name="xstage", bufs=2))
    x_sb = resident.tile([128, NCC, t], BF16, tag="x")
    for cc in range(NCC):
        xs = xstage.tile([128, t], F32, tag="xs")
        nc.sync.dma_start(xs[:], xT[cc * 128:(cc + 1) * 128, :])
        nc.vector.tensor_copy(x_sb[:, cc, :], xs[:])

    # ---- QK^T projection + RoPE ----
    # qk chunks: 0 = q heads(0,1), 1 = q heads(2,3), 2 = k(0,1), 3 = k(2,3)
    qk = resident.tile([128, 4, t], BF16, tag="qk")
    for jt in range(4):
        for tb in range(ntb):
            tsl = bass.ts(tb, QB)
            ps = psA.tile([128, QB], F32, tag="psA")
            for cc in range(NCC):
                nc.tensor.matmul(
                    ps[:],
                    wqk_sb[:, cc, bass.ts(jt, 128)],
                    x_sb[:, cc, tsl],
                    start=(cc == 0), stop=(cc == NCC - 1))
            raw = ropet.tile([128, QB], BF16, tag="raw")
            nc.vector.tensor_copy(raw[:], ps[:])
            rot = ropet.tile([128, QB], BF16, tag="rot")
            for s in range(4):
                nc.sync.dma_start(rot[s * 32:(s + 1) * 32, :],
                                  raw[(s ^ 1) * 32:((s ^ 1) + 1) * 32, :])
            cosp = ropet.tile([128, QB], BF16, tag="cosp")
            nc.vector.tensor_mul(cosp[:], raw[:], cos_sb[:, tsl])
            sinp = ropet.tile([128, QB], BF16, tag="sinp")
            nc.vector.tensor_mul(sinp[:], rot[:], sin_sb[:, tsl])
            nc.vector.tensor_add(qk[:, jt, tsl], cosp[:], sinp[:])

    # ---- V projection (natural layout, x stationary) ----
    # v layout [128, ntt*HG, 65]: per (t-tile, local head): 64 v cols + 1 ones
    v_sb = resident.tile([128, ntt * HG, HD + 1], BF16, tag="v")
    for tt in range(ntt):
        ps = psA.tile([128, HG * (HD + 1)], F32, tag="psA")
        for cc in range(NCC):
            nc.tensor.matmul(
                ps[:],
                x_sb[:, cc, bass.ts(tt, 128)],
                wv_sb[:, cc, :],
                start=(cc == 0), stop=(cc == NCC - 1))
        nc.vector.tensor_copy(
            v_sb[:, tt * HG:(tt + 1) * HG, :],
            ps.rearrange("p (h d) -> p h d", d=HD + 1))
    nc.vector.memset(v_sb[:, :, HD], 1.0)

    # ---- attention + output projection ----
    ynorm = resident.tile([128, 2, t], BF16, tag="ynorm")
    for qb in range(nqb):
        qsl = bass.ts(qb, QB)
        nkt = (qb + 1) * (QB // KT)
        for p in range(2):   # head pairs (0,1) and (2,3)
            qc = qk[:, p, :]
            kc = qk[:, 2 + p, :]
            ya = psY.tile([HD + 1, QB], F32, tag="psY")
            yb = psY.tile([HD + 1, QB], F32, tag="psY")
            ets = {}
            # software-pipelined: S^T/exp for kt runs ahead of V matmuls
            for kt in range(nkt + 1):
                if kt < nkt:
                    ksl = bass.ts(kt, KT)
                    pss = psS.tile([128, 2 * QB], F32, tag="psS")
                    nc.tensor.matmul(pss[:, 0:QB], kc[0:64, ksl],
                                     qc[0:64, qsl],
                                     start=True, stop=True,
                                     tile_position=(0, 0))
                    nc.tensor.matmul(pss[:, QB:2 * QB], kc[64:128, ksl],
                                     qc[64:128, qsl],
                                     start=True, stop=True,
                                     tile_position=(64, 0))
                    et = exps.tile([128, 2 * QB], BF16, tag="exps")
                    nc.scalar.activation(et[:], pss[:], Exp, scale=0.125)
                    if (kt + 1) * KT > qb * QB:  # diagonal: causal mask
                        nc.gpsimd.affine_select(
                            et.rearrange("p (h q) -> p h q", q=QB),
                            et.rearrange("p (h q) -> p h q", q=QB),
                            pattern=[[0, 2], [1, QB]], compare_op=ge,
                            fill=0.0,
                            base=qb * QB - kt * KT, channel_multiplier=-1)
                    ets[kt] = et
                if kt >= 1:
                    kv = kt - 1
                    et = ets.pop(kv)
                    first, last = (kv == 0), (kv == nkt - 1)
                    nc.tensor.matmul(ya[:], v_sb[:, kv * HG + 2 * p, :],
                                     et[:, 0:QB], start=first, stop=last)
                    nc.tensor.matmul(yb[:], v_sb[:, kv * HG + 2 * p + 1, :],
                                     et[:, QB:2 * QB],
                                     start=first, stop=last)
            # 1/sumexp = exp(-ln(s)) on ACT (DVE reciprocal is an 8-cycle-
            # per-element iterative divide — far too slow; gpsimd has no
            # tensor-tensor ALU on this ISA)
            sab = small.tile([1, 2 * QB], F32, tag="sab")
            nc.vector.tensor_copy(sab[:, 0:QB], ya[HD:HD + 1, :])
            nc.vector.tensor_copy(sab[:, QB:2 * QB], yb[HD:HD + 1, :])
            lab = small.tile([1, 2 * QB], F32, tag="lab")
            nc.scalar.activation(lab[:], sab[:], Log)
            rab = small.tile([1, 2 * QB], F32, tag="rab")
            nc.scalar.activation(rab[:], lab[:], Exp, scale=-1.0)
            for h01, yp in ((0, ya), (1, yb)):
                rb = small.tile([64, QB], F32, tag="rb")
                nc.gpsimd.partition_broadcast(rb[:], rab[:, h01 * QB:
                                                         (h01 + 1) * QB])
                nc.vector.tensor_mul(
                    ynorm[h01 * 64:(h01 + 1) * 64, p, qsl],
                    yp[0:HD, :], rb[:])
        # output projection for the 4 t-tiles of this q block
        for tt in range(qb * 4, qb * 4 + 4):
            for cb in range(2):
                ps = psA.tile([128, QB], F32, tag="psA")
                for jc in range(2):
                    nc.tensor.matmul(
                        ps[:], ynorm[:, jc, bass.ts(tt, 128)],
                        wp_sb[:, jc, bass.ts(cb, QB)],
                        start=(jc == 0), stop=(jc == 1))
                ot = outsb.tile([128, QB], F32, tag="ot")
                nc.vector.tensor_copy(ot[:], ps[:])
                nc.sync.dma_start(
                    out[tt * 128:(tt + 1) * 128, bass.ts(cb, QB)], ot[:])
    ctx.close()


def host_inputs(x, w_attn, w_proj, t=T):
    """Build the 8 per-core input maps from full inputs."""
    xTs = [np.ascontiguousarray(x[b, :t].T).astype(np.float32)
           for b in range(B)]
    inv = 1.0 / (10000.0 ** (np.arange(0, HD, 2, dtype=np.float32) / HD))
    fr = np.outer(np.arange(t, dtype=np.float32), inv)     # [t, 32]
    emb = np.concatenate([fr, fr], 1)                      # [t, 64]
    cos = np.cos(emb).T.astype(np.float32)                 # [64, t]
    sin = np.sin(emb).T.astype(np.float32)
    sin_s = sin.copy()
    sin_s[:32] *= -1.0
    cosT2 = np.tile(cos, (2, 1)).astype(BF)
    sinT2 = np.tile(sin_s, (2, 1)).astype(BF)

    in_maps = []
    for c in range(8):
        b, g = c // 4, c % 4
        wq = w_attn[g * GD:(g + 1) * GD]
        wk = w_attn[C + g * GD:C + (g + 1) * GD]
        wv = w_attn[2 * C + g * GD:2 * C + (g + 1) * GD]
        wqkT = np.ascontiguousarray(
            np.concatenate([wq, wk], 0).T).astype(BF)
        wvT = np.zeros((C, HG * (HD + 1)), BF)
        for h in range(HG):
            wvT[:, h * (HD + 1):h * (HD + 1) + HD] = \
                wv[h * HD:(h + 1) * HD].T.astype(BF)
        wpT = np.ascontiguousarray(
            w_proj[:, g * GD:(g + 1) * GD].T).astype(BF)
        in_maps.append({"xT": xTs[b], "wqkT": wqkT, "wvT": wvT,
                        "wpT": wpT, "cosT": cosT2, "sinT": sinT2})
    return in_maps


_cache = {}


def kernel(x, w_attn, w_proj):
    x = np.asarray(x, dtype=np.float32)
    w_attn = np.asarray(w_attn, dtype=np.float32)
    w_proj = np.asarray(w_proj, dtype=np.float32)
    if "nc" not in _cache:
        _cache["nc"] = build_kernel()
    nc = _cache["nc"]
    in_maps = host_inputs(x, w_attn, w_proj)
    res = run_bass_kernel_spmd(nc, in_maps, list(range(8)))
    out = np.zeros((B, T, C), dtype=np.float32)
    for c in range(8):
        out[c // 4] += res.results[c]["out"]
    return out
